# revision 7
# baseline (speedup 1.0000x reference)
"""BlockReLU Trainium2 kernel (8-core data-parallel over batch), fp16 device
I/O with per-group nested column de-interleaving.

Reference semantics (per [N, C, H, W] f32 input):
  channels  0:16  block (1,1): out = x * (x > 0)            == relu(x)
  channels 16:32  block (2,2): out = x * (mean_2x2(x) > 0)
  channels 32:48  block (4,4): out = x * (mean_4x4(x) > 0)
  channels 48:56  block (8,8): out = x * (mean_8x8(x) > 0)
  channels 56:64  identity

sign(mean) == sign(sum), so block sums replace means.  Device compute is
fp16 (correctness gate is rel_err < 2e-2; fp16 end-to-end measures
~2.7e-4), which halves HBM traffic and enables DVE 16-bit perf modes.
Identity channels are passed through on the host (exact f32).

Layout: per core the batch shard is packed to [112, HW] fp16, one
(channel, image) per dram row / SBUF partition:

  partitions  0:32   block (2,2) channels (c 16:32)
  partitions 32:64   block (4,4) channels (c 32:48)
  partitions 64:80   block (8,8) channels (c 48:56)
  partitions 96:128  block (1,1) channels (c  0:16)  (dram rows 80:112)

Within each row, columns are permuted host-side so that EVERY device op
is innermost-packed (measured: packed TT 0.60 ns/elem vs 1.1+ strided;
TS packed 0.34; broadcast only on non-innermost dims):

  level 1 (all block groups): [even cols | odd cols] — column-pair sums
    become two half-row adds, and the masked multiply becomes
    (row-pair, half, 96) with the quarter-res mask broadcast over the
    *middle* dim, innermost packed.
  level 2 (g4, g8): pair-index de-interleaved again — 4x4 column sums
    are again half-adds, and g4 mask expansion is msml[v] = m4[v % 48]
    (pure packed copies).
  level 3 (g8): once more — 8x8 sums are half-adds and g8 expansion is
    msml[v] = m8[v % 24].

Work split, balanced to the measured rates (DVE 0.6/0.34, ACT 0.97
pattern-insensitive, GpSimd 2.1 flat, DMA ~40-50us for 16.5 MB/core):
  DVE: all block-sum adds, is_gt masks, multiply row-parity 0, ~30% of
    relu rows (tensor_scalar max).
  ACT: mask expansions (copies) + ~70% of relu rows.
  GpSimd: multiply row-parity 1.
  DMA via SP HWDGE, 2 transfers per chunk per direction.
"""

import json
import re

import numpy as np

N, C, H, W = 16, 64, 192, 192
NCORES = 8
NB = N // NCORES  # batch per core
HW = H * W
C2 = 56  # channels computed on device (identity 56:64 handled on host)
DROWS = C2 * NB  # dram rows per core

CHUNK_ROWS = [32, 32, 32, 32, 32, 32]  # rows per chunk (each mult of 8)
# 32 rows -> 12.3KB per-partition DMA descriptors (18.4KB descriptors
# measured ~2x slower per byte on this part)
assert sum(CHUNK_ROWS) == H

XT_BUFS = len(CHUNK_ROWS)  # all loads are issued up-front
MSML_BUFS = len(CHUNK_ROWS)  # one per chunk: no ring-reuse waits
TMP_BUFS = 3
PIPE_DEPTH = 2

# relu row split: ScalarE / GpSimd (off the mask-mult critical chain) / DVE
RELU_ACT_ROWS_FRAC = 0.55
RELU_GPS_ROWS_FRAC = 0.30
MULT_DH1_GPS_FRAC = 0.0  # keep GpSimd off the mult critical chain

# device channel order: g2, g4, g8, then relu block
PERM2 = list(range(16, 56)) + list(range(0, 16))


def _deint(a):
    return np.concatenate([a[0::2], a[1::2]])


def _col_perms():
    """Per-group column permutation: packed[:, u] = x[:, perm[u]]."""
    ar = np.arange(W)
    p1 = _deint(ar)  # level 1: [even | odd]
    q = _deint(np.arange(W // 2))
    p2 = np.concatenate([p1[:96][q], p1[96:][q]])  # level 2 within halves
    r = _deint(np.arange(48))
    q8 = np.concatenate([2 * r, 2 * r + 1])
    p3 = np.concatenate([p1[:96][q8], p1[96:][q8]])  # level 3 variant
    return {
        "g2": p1,
        "g4": p2,
        "g8": p3,
        "g1": ar,
    }


_PC = _col_perms()
# per-device-channel-row column perm: rows 0:16 g2, 16:32 g4, 32:40 g8,
# 40:56 g1
_ROWGROUP = ["g2"] * 16 + ["g4"] * 16 + ["g8"] * 8 + ["g1"] * 16
COLP = np.stack([_PC[g] for g in _ROWGROUP])  # [56, 192]
ICOLP = np.argsort(COLP, axis=1)  # inverse perms

_CACHE = {}


def _split_multi_waits(bir_json: bytes) -> bytes:
    """This walrus build rejects >1 sync-wait per instruction; hoist extra
    waits onto fresh single-wait NoOps on the same engine."""
    m = json.loads(bir_json)
    max_idx = 0
    for f in m.get("functions", []):
        for b in f.get("blocks", []):
            for ins in b.get("instructions", []):
                mt = re.match(r"I-(\d+)$", ins.get("name", ""))
                if mt:
                    max_idx = max(max_idx, int(mt.group(1)))
    next_idx = max_idx + 1
    for f in m.get("functions", []):
        for b in f.get("blocks", []):
            out = []
            for ins in b.get("instructions", []):
                si = ins.get("sync_info")
                waits = (si or {}).get("on_wait") or []
                if len(waits) > 1:
                    for w in waits[:-1]:
                        out.append(
                            {
                                "debug": ins.get("debug"),
                                "engine": ins["engine"],
                                "ins": [],
                                "name": f"I-{next_idx}",
                                "opcode": "NoOp",
                                "outs": [],
                                "sync_info": {"on_wait": [w], "on_update": []},
                            }
                        )
                        next_idx += 1
                    si["on_wait"] = [waits[-1]]
                out.append(ins)
            b["instructions"] = out
    return json.dumps(m).encode()


def _install_birpatch():
    import concourse.bass2jax as b2j
    import concourse.bass_utils as bu

    if getattr(bu, "_split_waits_installed", False):
        return
    orig = bu.compile_bir_kernel

    def compile_bir_kernel_split(bir_json, tmpdir, neff_name="file.neff"):
        return orig(_split_multi_waits(bir_json), tmpdir, neff_name)

    bu.compile_bir_kernel = compile_bir_kernel_split
    b2j.compile_bir_kernel = compile_bir_kernel_split
    bu._split_waits_installed = True


def _build_nc():
    import concourse.bass as bass
    import concourse.mybir as mybir
    from concourse.tile import TileContext

    _install_birpatch()

    f16 = mybir.dt.float16
    ALU = mybir.AluOpType
    AF = mybir.ActivationFunctionType

    nc = bass.Bass("TRN2", debug=False)
    xs = nc.dram_tensor("x", [DROWS, HW], f16, kind="ExternalInput").ap()
    ys = nc.dram_tensor("y", [DROWS, HW], f16, kind="ExternalOutput").ap()

    RMAX = max(CHUNK_ROWS)
    LMAX = RMAX * W

    with TileContext(nc) as tc:
        with (
            tc.tile_pool(name="xt", bufs=XT_BUFS) as px,
            tc.tile_pool(name="mm", bufs=MSML_BUFS) as pmm,
            tc.tile_pool(name="tmp", bufs=TMP_BUFS) as pt,
        ):

            def emit_mult(xt, msml, row0, rows):
                """masked multiply + store, PIPE_DEPTH chunks behind."""
                lc = rows * W
                r2 = rows // 2

                # masked multiply [0:80]: (r2, parity, half, 96), mask
                # broadcast over parity (sub-op dim) and half (middle dim)
                vx = xt[0:80, :lc].rearrange(
                    "p (r t h a) -> p r t h a", t=2, h=2, a=96
                )
                mbf = msml[0:80, : lc // 4].rearrange("p (r a) -> p r a", a=96)

                def mult(eng, dh, a, b):
                    if a >= b:
                        return
                    o = vx[:, a:b, dh, :, :]
                    m = (
                        mbf[:, a:b, :]
                        .unsqueeze(2)
                        .broadcast_to([80, b - a, 2, 96])
                    )
                    eng.tensor_tensor(out=o, in0=o, in1=m, op=ALU.mult)

                ug = int(round(MULT_DH1_GPS_FRAC * r2))
                mult(nc.vector, 0, 0, r2)
                if ug:
                    mult(nc.gpsimd, 1, 0, ug)
                mult(nc.vector, 1, ug, r2)

                nc.sync.dma_start(
                    out=ys[0:80, row0 * W : row0 * W + lc], in_=xt[0:80, :lc]
                )
                nc.sync.dma_start(
                    out=ys[80:112, row0 * W : row0 * W + lc],
                    in_=xt[96:128, :lc],
                )

            # issue ALL loads up-front so the in-order SP DMA queue never
            # head-blocks later loads behind mult-gated stores
            xts = []
            row0 = 0
            for rows in CHUNK_ROWS:
                lc = rows * W
                xt = px.tile([128, LMAX], f16, tag="xt")
                cols = slice(row0 * W, row0 * W + lc)
                nc.sync.dma_start(out=xt[0:80, :lc], in_=xs[0:80, cols])
                nc.sync.dma_start(out=xt[96:128, :lc], in_=xs[80:112, cols])
                xts.append(xt)
                row0 += rows

            pending = []
            row0 = 0
            for ci, rows in enumerate(CHUNK_ROWS):
                lc = rows * W
                r2, r4, r8 = rows // 2, rows // 4, rows // 8
                xt = xts[ci]
                msml = pmm.tile([80, LMAX // 4], f16, tag="msml")
                t1 = pt.tile([80, LMAX // 2], f16, tag="t1")
                sa = pt.tile([80, LMAX // 4], f16, tag="sa")
                t2 = pt.tile([80, LMAX // 8], f16, tag="t2")
                sb = pt.tile([80, LMAX // 16], f16, tag="sb")
                m4 = pt.tile([80, LMAX // 16], f16, tag="m4")
                t3 = pt.tile([80, LMAX // 32], f16, tag="t3")
                sc = pt.tile([80, LMAX // 64], f16, tag="sc")

                # relu [96:128] up-front: depends only on this chunk's load;
                # row-split ACT / GpSimd / DVE (GpSimd is otherwise idle and
                # off the mask->mult critical chain)
                sr = int(round(RELU_ACT_ROWS_FRAC * rows))
                sg = sr + int(round(RELU_GPS_ROWS_FRAC * rows))
                nc.scalar.activation(
                    out=xt[96:128, : sr * W], in_=xt[96:128, : sr * W],
                    func=AF.Relu,
                )
                if sg > sr:
                    nc.gpsimd.tensor_scalar(
                        xt[96:128, sr * W : sg * W], xt[96:128, sr * W : sg * W],
                        0.0, None, ALU.max,
                    )
                if sg < rows:
                    nc.vector.tensor_scalar(
                        xt[96:128, sg * W : lc], xt[96:128, sg * W : lc],
                        0.0, None, ALU.max,
                    )

                V = nc.vector

                def rowpair(src, dst, p0, p1, w, r):
                    """dst (r/2, w) = row-pair sums of src (r, w); packed."""
                    vr = src[p0:p1, : r * w].rearrange(
                        "p (r t a) -> p r t a", t=2, a=w
                    )
                    V.tensor_tensor(
                        out=dst[p0:p1, : r * w // 2].rearrange(
                            "p (r a) -> p r a", a=w
                        ),
                        in0=vr[:, :, 0, :], in1=vr[:, :, 1, :], op=ALU.add)

                def halfadd(src, dst, p0, p1, w, r):
                    """dst (r, w/2) = src[:, :w/2] + src[:, w/2:]; packed."""
                    vh = src[p0:p1, : r * w].rearrange(
                        "p (r h a) -> p r h a", h=2, a=w // 2
                    )
                    V.tensor_tensor(
                        out=dst[p0:p1, : r * w // 2].rearrange(
                            "p (r a) -> p r a", a=w // 2
                        ),
                        in0=vh[:, :, 0, :], in1=vh[:, :, 1, :], op=ALU.add)

                # 2x2 sums [0:80]
                rowpair(xt, t1, 0, 80, W, rows)
                halfadd(t1, sa, 0, 80, W, r2)
                nc.vector.tensor_scalar(
                    msml[0:32, : lc // 4], sa[0:32, : lc // 4], 0.0, None,
                    ALU.is_gt,
                )
                # 4x4 sums ([0:32] wasted but merged op is free-size bound)
                rowpair(sa, t2, 0, 80, 96, r2)
                halfadd(t2, sb, 0, 80, 96, r4)
                nc.vector.tensor_scalar(
                    m4[32:64, : lc // 16], sb[32:64, : lc // 16], 0.0, None,
                    ALU.is_gt,
                )
                # 8x8 sums (g8 only)
                rowpair(sb, t3, 64, 80, 48, r4)
                halfadd(t3, sc, 64, 80, 48, r8)
                nc.vector.tensor_scalar(
                    sc[64:80, : lc // 64], sc[64:80, : lc // 64], 0.0, None,
                    ALU.is_gt,
                )

                # g4 expansion: msml[v] = m4[v % 48], per row-parity (ACT)
                vg4 = msml[32:64, : lc // 4].rearrange(
                    "p (r d a) -> p r d a", d=2, a=96
                )
                m4v = m4[32:64, : lc // 16].rearrange("p (r t) -> p r t", t=48)
                for dr in range(2):
                    nc.scalar.copy(
                        out=vg4[:, :, dr, :].rearrange(
                            "p r (u t) -> p r u t", t=48
                        ),
                        in_=m4v.unsqueeze(2).broadcast_to([32, r4, 2, 48]),
                    )
                # g8 expansion: msml[v] = m8[v % 24], per row-parity (ACT)
                vg8 = msml[64:80, : lc // 4].rearrange(
                    "p (r d a) -> p r d a", d=4, a=96
                )
                m8v = sc[64:80, : lc // 64].rearrange("p (r t) -> p r t", t=24)
                for dr in range(4):
                    nc.scalar.copy(
                        out=vg8[:, :, dr, :].rearrange(
                            "p r (q t) -> p r q t", t=24
                        ),
                        in_=m8v.unsqueeze(2).broadcast_to([16, r8, 4, 24]),
                    )

                pending.append((xt, msml, row0, rows))
                if len(pending) > PIPE_DEPTH:
                    emit_mult(*pending.pop(0))
                row0 += rows

            while pending:
                emit_mult(*pending.pop(0))

    return nc


def make_in_maps(activation: np.ndarray) -> list[dict]:
    """Per-core device inputs: [112, HW] fp16, channels PERM2, per-group
    column de-interleave, (c n) row order."""
    gi = np.arange(C2)[:, None, None]
    ci = COLP[:, None, :]
    maps = []
    for k in range(NCORES):
        a = activation[k * NB : (k + 1) * NB][:, PERM2]  # [2, 56, H, W]
        a = a[:, gi, np.arange(H)[None, :, None], ci]  # per-group col perm
        a = np.ascontiguousarray(a.transpose(1, 0, 2, 3).astype(np.float16))
        maps.append({"x": a.reshape(DROWS, HW)})
    return maps


def kernel(activation: np.ndarray) -> np.ndarray:
    from concourse import bass_utils

    activation = np.asarray(activation)
    assert activation.shape == (N, C, H, W) and activation.dtype == np.float32

    if "nc" not in _CACHE:
        _CACHE["nc"] = _build_nc()
    nc = _CACHE["nc"]

    in_maps = make_in_maps(activation)
    res = bass_utils.run_bass_kernel_spmd(nc, in_maps, core_ids=list(range(NCORES)))
    out = np.empty((N, C, H, W), dtype=np.float32)
    gi = np.arange(C2)[:, None, None]
    ici = ICOLP[:, None, :]
    for k in range(NCORES):
        yk = (
            np.asarray(res.results[k]["y"])
            .reshape(C2, NB, H, W)
            .transpose(1, 0, 2, 3)
            .astype(np.float32)
        )
        yk = yk[:, gi, np.arange(H)[None, :, None], ici]  # undo col perm
        out[k * NB : (k + 1) * NB, PERM2] = yk
    out[:, 56:64] = activation[:, 56:64]  # identity channels: exact f32
    return out


# revision 8
# speedup vs baseline: 2.7119x; 2.7119x over previous
"""BlockReLU Trainium2 kernel (8-core data-parallel over batch), fp16 device
I/O with per-group nested column de-interleaving.

Reference semantics (per [N, C, H, W] f32 input):
  channels  0:16  block (1,1): out = x * (x > 0)            == relu(x)
  channels 16:32  block (2,2): out = x * (mean_2x2(x) > 0)
  channels 32:48  block (4,4): out = x * (mean_4x4(x) > 0)
  channels 48:56  block (8,8): out = x * (mean_8x8(x) > 0)
  channels 56:64  identity

sign(mean) == sign(sum), so block sums replace means.  Device compute is
fp16 (correctness gate is rel_err < 2e-2; fp16 end-to-end measures
~2.7e-4), which halves HBM traffic and enables DVE 16-bit perf modes.
Identity channels are passed through on the host (exact f32).

Layout: per core the batch shard is packed to [112, HW] fp16, one
(channel, image) per dram row / SBUF partition:

  partitions  0:32   block (2,2) channels (c 16:32)
  partitions 32:64   block (4,4) channels (c 32:48)
  partitions 64:80   block (8,8) channels (c 48:56)
  partitions 96:128  block (1,1) channels (c  0:16)  (dram rows 80:112)

Within each row, columns are permuted host-side so that EVERY device op
is innermost-packed (measured: packed TT 0.60 ns/elem vs 1.1+ strided;
TS packed 0.34; broadcast only on non-innermost dims):

  level 1 (all block groups): [even cols | odd cols] — column-pair sums
    become two half-row adds, and the masked multiply becomes
    (row-pair, half, 96) with the quarter-res mask broadcast over the
    *middle* dim, innermost packed.
  level 2 (g4, g8): pair-index de-interleaved again — 4x4 column sums
    are again half-adds, and g4 mask expansion is msml[v] = m4[v % 48]
    (pure packed copies).
  level 3 (g8): once more — 8x8 sums are half-adds and g8 expansion is
    msml[v] = m8[v % 24].

Work split, balanced to the measured rates (DVE 0.6/0.34, ACT 0.97
pattern-insensitive, GpSimd 2.1 flat, DMA ~40-50us for 16.5 MB/core):
  DVE: all block-sum adds, is_gt masks, multiply row-parity 0, ~30% of
    relu rows (tensor_scalar max).
  ACT: mask expansions (copies) + ~70% of relu rows.
  GpSimd: multiply row-parity 1.
  DMA via SP HWDGE, 2 transfers per chunk per direction.
"""

import json
import re

import numpy as np

N, C, H, W = 16, 64, 192, 192
NCORES = 8
NB = N // NCORES  # batch per core
HW = H * W
C2 = 56  # channels computed on device (identity 56:64 handled on host)
DROWS = C2 * NB  # dram rows per core

CHUNK_ROWS = [32, 32, 32, 32, 32, 32]  # rows per chunk (each mult of 8)
# 32 rows -> 12.3KB per-partition DMA descriptors (18.4KB descriptors
# measured ~2x slower per byte on this part)
assert sum(CHUNK_ROWS) == H

XT_BUFS = len(CHUNK_ROWS)  # all loads are issued up-front
MSML_BUFS = len(CHUNK_ROWS)  # one per chunk: no ring-reuse waits
TMP_BUFS = 3
PIPE_DEPTH = 2

# relu row split: ScalarE / GpSimd (off the mask-mult critical chain) / DVE
RELU_ACT_ROWS_FRAC = 0.65
RELU_GPS_ROWS_FRAC = 0.0  # GpSimd tensor_scalar measured ~15ns/elem: unusable
MULT_DH1_GPS_FRAC = 0.0  # keep GpSimd off the mult critical chain

# device channel order: g2, g4, g8, then relu block
PERM2 = list(range(16, 56)) + list(range(0, 16))


def _deint(a):
    return np.concatenate([a[0::2], a[1::2]])


def _col_perms():
    """Per-group column permutation: packed[:, u] = x[:, perm[u]]."""
    ar = np.arange(W)
    p1 = _deint(ar)  # level 1: [even | odd]
    q = _deint(np.arange(W // 2))
    p2 = np.concatenate([p1[:96][q], p1[96:][q]])  # level 2 within halves
    r = _deint(np.arange(48))
    q8 = np.concatenate([2 * r, 2 * r + 1])
    p3 = np.concatenate([p1[:96][q8], p1[96:][q8]])  # level 3 variant
    return {
        "g2": p1,
        "g4": p2,
        "g8": p3,
        "g1": ar,
    }


_PC = _col_perms()
# per-device-channel-row column perm: rows 0:16 g2, 16:32 g4, 32:40 g8,
# 40:56 g1
_ROWGROUP = ["g2"] * 16 + ["g4"] * 16 + ["g8"] * 8 + ["g1"] * 16
COLP = np.stack([_PC[g] for g in _ROWGROUP])  # [56, 192]
ICOLP = np.argsort(COLP, axis=1)  # inverse perms

_CACHE = {}


def _split_multi_waits(bir_json: bytes) -> bytes:
    """This walrus build rejects >1 sync-wait per instruction; hoist extra
    waits onto fresh single-wait NoOps on the same engine."""
    m = json.loads(bir_json)
    max_idx = 0
    for f in m.get("functions", []):
        for b in f.get("blocks", []):
            for ins in b.get("instructions", []):
                mt = re.match(r"I-(\d+)$", ins.get("name", ""))
                if mt:
                    max_idx = max(max_idx, int(mt.group(1)))
    next_idx = max_idx + 1
    for f in m.get("functions", []):
        for b in f.get("blocks", []):
            out = []
            for ins in b.get("instructions", []):
                si = ins.get("sync_info")
                waits = (si or {}).get("on_wait") or []
                if len(waits) > 1:
                    for w in waits[:-1]:
                        out.append(
                            {
                                "debug": ins.get("debug"),
                                "engine": ins["engine"],
                                "ins": [],
                                "name": f"I-{next_idx}",
                                "opcode": "NoOp",
                                "outs": [],
                                "sync_info": {"on_wait": [w], "on_update": []},
                            }
                        )
                        next_idx += 1
                    si["on_wait"] = [waits[-1]]
                out.append(ins)
            b["instructions"] = out
    return json.dumps(m).encode()


def _install_birpatch():
    import concourse.bass2jax as b2j
    import concourse.bass_utils as bu

    if getattr(bu, "_split_waits_installed", False):
        return
    orig = bu.compile_bir_kernel

    def compile_bir_kernel_split(bir_json, tmpdir, neff_name="file.neff"):
        return orig(_split_multi_waits(bir_json), tmpdir, neff_name)

    bu.compile_bir_kernel = compile_bir_kernel_split
    b2j.compile_bir_kernel = compile_bir_kernel_split
    bu._split_waits_installed = True


def _build_nc():
    import concourse.bass as bass
    import concourse.mybir as mybir
    from concourse.tile import TileContext

    _install_birpatch()

    f16 = mybir.dt.float16
    ALU = mybir.AluOpType
    AF = mybir.ActivationFunctionType

    nc = bass.Bass("TRN2", debug=False)
    xs = nc.dram_tensor("x", [DROWS, HW], f16, kind="ExternalInput").ap()
    ys = nc.dram_tensor("y", [DROWS, HW], f16, kind="ExternalOutput").ap()

    RMAX = max(CHUNK_ROWS)
    LMAX = RMAX * W

    with TileContext(nc) as tc:
        with (
            tc.tile_pool(name="xt", bufs=XT_BUFS) as px,
            tc.tile_pool(name="mm", bufs=MSML_BUFS) as pmm,
            tc.tile_pool(name="tmp", bufs=TMP_BUFS) as pt,
        ):

            def emit_mult(xt, msml, row0, rows):
                """masked multiply + store, PIPE_DEPTH chunks behind."""
                lc = rows * W
                r2 = rows // 2

                # masked multiply [0:80]: (r2, parity, half, 96), mask
                # broadcast over parity (sub-op dim) and half (middle dim)
                vx = xt[0:80, :lc].rearrange(
                    "p (r t h a) -> p r t h a", t=2, h=2, a=96
                )
                mbf = msml[0:80, : lc // 4].rearrange("p (r a) -> p r a", a=96)

                def mult(eng, dh, a, b):
                    if a >= b:
                        return
                    o = vx[:, a:b, dh, :, :]
                    m = (
                        mbf[:, a:b, :]
                        .unsqueeze(2)
                        .broadcast_to([80, b - a, 2, 96])
                    )
                    eng.tensor_tensor(out=o, in0=o, in1=m, op=ALU.mult)

                ug = int(round(MULT_DH1_GPS_FRAC * r2))
                mult(nc.vector, 0, 0, r2)
                if ug:
                    mult(nc.gpsimd, 1, 0, ug)
                mult(nc.vector, 1, ug, r2)

                nc.sync.dma_start(
                    out=ys[0:80, row0 * W : row0 * W + lc], in_=xt[0:80, :lc]
                )
                nc.sync.dma_start(
                    out=ys[80:112, row0 * W : row0 * W + lc],
                    in_=xt[96:128, :lc],
                )

            # issue ALL loads up-front so the in-order SP DMA queue never
            # head-blocks later loads behind mult-gated stores
            xts = []
            row0 = 0
            for rows in CHUNK_ROWS:
                lc = rows * W
                xt = px.tile([128, LMAX], f16, tag="xt")
                cols = slice(row0 * W, row0 * W + lc)
                nc.sync.dma_start(out=xt[0:80, :lc], in_=xs[0:80, cols])
                nc.sync.dma_start(out=xt[96:128, :lc], in_=xs[80:112, cols])
                xts.append(xt)
                row0 += rows

            pending = []
            row0 = 0
            for ci, rows in enumerate(CHUNK_ROWS):
                lc = rows * W
                r2, r4, r8 = rows // 2, rows // 4, rows // 8
                xt = xts[ci]
                msml = pmm.tile([80, LMAX // 4], f16, tag="msml")
                t1 = pt.tile([80, LMAX // 2], f16, tag="t1")
                sa = pt.tile([80, LMAX // 4], f16, tag="sa")
                t2 = pt.tile([80, LMAX // 8], f16, tag="t2")
                sb = pt.tile([80, LMAX // 16], f16, tag="sb")
                m4 = pt.tile([80, LMAX // 16], f16, tag="m4")
                t3 = pt.tile([80, LMAX // 32], f16, tag="t3")
                sc = pt.tile([80, LMAX // 64], f16, tag="sc")

                # relu [96:128] up-front: depends only on this chunk's load;
                # row-split ACT / GpSimd / DVE (GpSimd is otherwise idle and
                # off the mask->mult critical chain)
                sr = int(round(RELU_ACT_ROWS_FRAC * rows))
                sg = sr + int(round(RELU_GPS_ROWS_FRAC * rows))
                nc.scalar.activation(
                    out=xt[96:128, : sr * W], in_=xt[96:128, : sr * W],
                    func=AF.Relu,
                )
                if sg > sr:
                    nc.gpsimd.tensor_scalar(
                        xt[96:128, sr * W : sg * W], xt[96:128, sr * W : sg * W],
                        0.0, None, ALU.max,
                    )
                if sg < rows:
                    nc.vector.tensor_scalar(
                        xt[96:128, sg * W : lc], xt[96:128, sg * W : lc],
                        0.0, None, ALU.max,
                    )

                V = nc.vector

                def rowpair(src, dst, p0, p1, w, r):
                    """dst (r/2, w) = row-pair sums of src (r, w); packed."""
                    vr = src[p0:p1, : r * w].rearrange(
                        "p (r t a) -> p r t a", t=2, a=w
                    )
                    V.tensor_tensor(
                        out=dst[p0:p1, : r * w // 2].rearrange(
                            "p (r a) -> p r a", a=w
                        ),
                        in0=vr[:, :, 0, :], in1=vr[:, :, 1, :], op=ALU.add)

                def halfadd(src, dst, p0, p1, w, r):
                    """dst (r, w/2) = src[:, :w/2] + src[:, w/2:]; packed."""
                    vh = src[p0:p1, : r * w].rearrange(
                        "p (r h a) -> p r h a", h=2, a=w // 2
                    )
                    V.tensor_tensor(
                        out=dst[p0:p1, : r * w // 2].rearrange(
                            "p (r a) -> p r a", a=w // 2
                        ),
                        in0=vh[:, :, 0, :], in1=vh[:, :, 1, :], op=ALU.add)

                # 2x2 sums [0:80]
                rowpair(xt, t1, 0, 80, W, rows)
                halfadd(t1, sa, 0, 80, W, r2)
                nc.vector.tensor_scalar(
                    msml[0:32, : lc // 4], sa[0:32, : lc // 4], 0.0, None,
                    ALU.is_gt,
                )
                # 4x4 sums ([0:32] wasted but merged op is free-size bound)
                rowpair(sa, t2, 0, 80, 96, r2)
                halfadd(t2, sb, 0, 80, 96, r4)
                nc.vector.tensor_scalar(
                    m4[32:64, : lc // 16], sb[32:64, : lc // 16], 0.0, None,
                    ALU.is_gt,
                )
                # 8x8 sums (g8 only)
                rowpair(sb, t3, 64, 80, 48, r4)
                halfadd(t3, sc, 64, 80, 48, r8)
                nc.vector.tensor_scalar(
                    sc[64:80, : lc // 64], sc[64:80, : lc // 64], 0.0, None,
                    ALU.is_gt,
                )

                # g4 expansion: msml[v] = m4[v % 48], per row-parity (ACT)
                vg4 = msml[32:64, : lc // 4].rearrange(
                    "p (r d a) -> p r d a", d=2, a=96
                )
                m4v = m4[32:64, : lc // 16].rearrange("p (r t) -> p r t", t=48)
                for dr in range(2):
                    nc.scalar.copy(
                        out=vg4[:, :, dr, :].rearrange(
                            "p r (u t) -> p r u t", t=48
                        ),
                        in_=m4v.unsqueeze(2).broadcast_to([32, r4, 2, 48]),
                    )
                # g8 expansion: msml[v] = m8[v % 24], per row-parity (ACT)
                vg8 = msml[64:80, : lc // 4].rearrange(
                    "p (r d a) -> p r d a", d=4, a=96
                )
                m8v = sc[64:80, : lc // 64].rearrange("p (r t) -> p r t", t=24)
                for dr in range(4):
                    nc.scalar.copy(
                        out=vg8[:, :, dr, :].rearrange(
                            "p r (q t) -> p r q t", t=24
                        ),
                        in_=m8v.unsqueeze(2).broadcast_to([16, r8, 4, 24]),
                    )

                pending.append((xt, msml, row0, rows))
                if len(pending) > PIPE_DEPTH:
                    emit_mult(*pending.pop(0))
                row0 += rows

            while pending:
                emit_mult(*pending.pop(0))

    return nc


def make_in_maps(activation: np.ndarray) -> list[dict]:
    """Per-core device inputs: [112, HW] fp16, channels PERM2, per-group
    column de-interleave, (c n) row order."""
    gi = np.arange(C2)[:, None, None]
    ci = COLP[:, None, :]
    maps = []
    for k in range(NCORES):
        a = activation[k * NB : (k + 1) * NB][:, PERM2]  # [2, 56, H, W]
        a = a[:, gi, np.arange(H)[None, :, None], ci]  # per-group col perm
        a = np.ascontiguousarray(a.transpose(1, 0, 2, 3).astype(np.float16))
        maps.append({"x": a.reshape(DROWS, HW)})
    return maps


def kernel(activation: np.ndarray) -> np.ndarray:
    from concourse import bass_utils

    activation = np.asarray(activation)
    assert activation.shape == (N, C, H, W) and activation.dtype == np.float32

    if "nc" not in _CACHE:
        _CACHE["nc"] = _build_nc()
    nc = _CACHE["nc"]

    in_maps = make_in_maps(activation)
    res = bass_utils.run_bass_kernel_spmd(nc, in_maps, core_ids=list(range(NCORES)))
    out = np.empty((N, C, H, W), dtype=np.float32)
    gi = np.arange(C2)[:, None, None]
    ici = ICOLP[:, None, :]
    for k in range(NCORES):
        yk = (
            np.asarray(res.results[k]["y"])
            .reshape(C2, NB, H, W)
            .transpose(1, 0, 2, 3)
            .astype(np.float32)
        )
        yk = yk[:, gi, np.arange(H)[None, :, None], ici]  # undo col perm
        out[k * NB : (k + 1) * NB, PERM2] = yk
    out[:, 56:64] = activation[:, 56:64]  # identity channels: exact f32
    return out


# revision 9
# speedup vs baseline: 2.7749x; 1.0232x over previous
"""BlockReLU Trainium2 kernel (8-core data-parallel over batch), fp16 device
I/O with per-group nested column de-interleaving.

Reference semantics (per [N, C, H, W] f32 input):
  channels  0:16  block (1,1): out = x * (x > 0)            == relu(x)
  channels 16:32  block (2,2): out = x * (mean_2x2(x) > 0)
  channels 32:48  block (4,4): out = x * (mean_4x4(x) > 0)
  channels 48:56  block (8,8): out = x * (mean_8x8(x) > 0)
  channels 56:64  identity

sign(mean) == sign(sum), so block sums replace means.  Device compute is
fp16 (correctness gate is rel_err < 2e-2; fp16 end-to-end measures
~2.7e-4), which halves HBM traffic and enables DVE 16-bit perf modes.
Identity channels are passed through on the host (exact f32).

Layout: per core the batch shard is packed to [112, HW] fp16, one
(channel, image) per dram row / SBUF partition:

  partitions  0:32   block (2,2) channels (c 16:32)
  partitions 32:64   block (4,4) channels (c 32:48)
  partitions 64:80   block (8,8) channels (c 48:56)
  partitions 96:128  block (1,1) channels (c  0:16)  (dram rows 80:112)

Within each row, columns are permuted host-side so that EVERY device op
is innermost-packed (measured: packed TT 0.60 ns/elem vs 1.1+ strided;
TS packed 0.34; broadcast only on non-innermost dims):

  level 1 (all block groups): [even cols | odd cols] — column-pair sums
    become two half-row adds, and the masked multiply becomes
    (row-pair, half, 96) with the quarter-res mask broadcast over the
    *middle* dim, innermost packed.
  level 2 (g4, g8): pair-index de-interleaved again — 4x4 column sums
    are again half-adds, and g4 mask expansion is msml[v] = m4[v % 48]
    (pure packed copies).
  level 3 (g8): once more — 8x8 sums are half-adds and g8 expansion is
    msml[v] = m8[v % 24].

Work split, balanced to the measured rates (DVE 0.6/0.34, ACT 0.97
pattern-insensitive, GpSimd 2.1 flat, DMA ~40-50us for 16.5 MB/core):
  DVE: all block-sum adds, is_gt masks, multiply row-parity 0, ~30% of
    relu rows (tensor_scalar max).
  ACT: mask expansions (copies) + ~70% of relu rows.
  GpSimd: multiply row-parity 1.
  DMA via SP HWDGE, 2 transfers per chunk per direction.
"""

import json
import re

import numpy as np

N, C, H, W = 16, 64, 192, 192
NCORES = 8
NB = N // NCORES  # batch per core
HW = H * W
C2 = 56  # channels computed on device (identity 56:64 handled on host)
DROWS = C2 * NB  # dram rows per core

CHUNK_ROWS = [16, 32, 32, 32, 32, 32, 16]  # rows per chunk (each mult of 8)
# 32 rows -> 12.3KB per-partition DMA descriptors (18.4KB descriptors
# measured ~2x slower per byte on this part)
assert sum(CHUNK_ROWS) == H

XT_BUFS = len(CHUNK_ROWS)  # all loads are issued up-front
MSML_BUFS = len(CHUNK_ROWS)  # one per chunk: no ring-reuse waits
TMP_BUFS = 3
PIPE_DEPTH = 2

# relu row split: ScalarE / GpSimd (off the mask-mult critical chain) / DVE
RELU_ACT_ROWS_FRAC = 0.65
RELU_GPS_ROWS_FRAC = 0.0  # GpSimd tensor_scalar measured ~15ns/elem: unusable
MULT_DH1_GPS_FRAC = 0.0  # keep GpSimd off the mult critical chain

# device channel order: g2, g4, g8, then relu block
PERM2 = list(range(16, 56)) + list(range(0, 16))


def _deint(a):
    return np.concatenate([a[0::2], a[1::2]])


def _col_perms():
    """Per-group column permutation: packed[:, u] = x[:, perm[u]]."""
    ar = np.arange(W)
    p1 = _deint(ar)  # level 1: [even | odd]
    q = _deint(np.arange(W // 2))
    p2 = np.concatenate([p1[:96][q], p1[96:][q]])  # level 2 within halves
    r = _deint(np.arange(48))
    q8 = np.concatenate([2 * r, 2 * r + 1])
    p3 = np.concatenate([p1[:96][q8], p1[96:][q8]])  # level 3 variant
    return {
        "g2": p1,
        "g4": p2,
        "g8": p3,
        "g1": ar,
    }


_PC = _col_perms()
# per-device-channel-row column perm: rows 0:16 g2, 16:32 g4, 32:40 g8,
# 40:56 g1
_ROWGROUP = ["g2"] * 16 + ["g4"] * 16 + ["g8"] * 8 + ["g1"] * 16
COLP = np.stack([_PC[g] for g in _ROWGROUP])  # [56, 192]
ICOLP = np.argsort(COLP, axis=1)  # inverse perms

_CACHE = {}


def _split_multi_waits(bir_json: bytes) -> bytes:
    """This walrus build rejects >1 sync-wait per instruction; hoist extra
    waits onto fresh single-wait NoOps on the same engine."""
    m = json.loads(bir_json)
    max_idx = 0
    for f in m.get("functions", []):
        for b in f.get("blocks", []):
            for ins in b.get("instructions", []):
                mt = re.match(r"I-(\d+)$", ins.get("name", ""))
                if mt:
                    max_idx = max(max_idx, int(mt.group(1)))
    next_idx = max_idx + 1
    for f in m.get("functions", []):
        for b in f.get("blocks", []):
            out = []
            for ins in b.get("instructions", []):
                si = ins.get("sync_info")
                waits = (si or {}).get("on_wait") or []
                if len(waits) > 1:
                    for w in waits[:-1]:
                        out.append(
                            {
                                "debug": ins.get("debug"),
                                "engine": ins["engine"],
                                "ins": [],
                                "name": f"I-{next_idx}",
                                "opcode": "NoOp",
                                "outs": [],
                                "sync_info": {"on_wait": [w], "on_update": []},
                            }
                        )
                        next_idx += 1
                    si["on_wait"] = [waits[-1]]
                out.append(ins)
            b["instructions"] = out
    return json.dumps(m).encode()


def _install_birpatch():
    import concourse.bass2jax as b2j
    import concourse.bass_utils as bu

    if getattr(bu, "_split_waits_installed", False):
        return
    orig = bu.compile_bir_kernel

    def compile_bir_kernel_split(bir_json, tmpdir, neff_name="file.neff"):
        return orig(_split_multi_waits(bir_json), tmpdir, neff_name)

    bu.compile_bir_kernel = compile_bir_kernel_split
    b2j.compile_bir_kernel = compile_bir_kernel_split
    bu._split_waits_installed = True


def _build_nc():
    import concourse.bass as bass
    import concourse.mybir as mybir
    from concourse.tile import TileContext

    _install_birpatch()

    f16 = mybir.dt.float16
    ALU = mybir.AluOpType
    AF = mybir.ActivationFunctionType

    nc = bass.Bass("TRN2", debug=False)
    xs = nc.dram_tensor("x", [DROWS, HW], f16, kind="ExternalInput").ap()
    ys = nc.dram_tensor("y", [DROWS, HW], f16, kind="ExternalOutput").ap()

    RMAX = max(CHUNK_ROWS)
    LMAX = RMAX * W

    with TileContext(nc) as tc:
        with (
            tc.tile_pool(name="xt", bufs=XT_BUFS) as px,
            tc.tile_pool(name="mm", bufs=MSML_BUFS) as pmm,
            tc.tile_pool(name="tmp", bufs=TMP_BUFS) as pt,
        ):

            def emit_mult(xt, msml, row0, rows):
                """masked multiply + store, PIPE_DEPTH chunks behind."""
                lc = rows * W
                r2 = rows // 2

                # masked multiply [0:80]: (r2, parity, half, 96), mask
                # broadcast over parity (sub-op dim) and half (middle dim)
                vx = xt[0:80, :lc].rearrange(
                    "p (r t h a) -> p r t h a", t=2, h=2, a=96
                )
                mbf = msml[0:80, : lc // 4].rearrange("p (r a) -> p r a", a=96)

                def mult(eng, dh, a, b):
                    if a >= b:
                        return
                    o = vx[:, a:b, dh, :, :]
                    m = (
                        mbf[:, a:b, :]
                        .unsqueeze(2)
                        .broadcast_to([80, b - a, 2, 96])
                    )
                    eng.tensor_tensor(out=o, in0=o, in1=m, op=ALU.mult)

                ug = int(round(MULT_DH1_GPS_FRAC * r2))
                mult(nc.vector, 0, 0, r2)
                if ug:
                    mult(nc.gpsimd, 1, 0, ug)
                mult(nc.vector, 1, ug, r2)

                # stores on the GpSimd SWDGE queue: independent of the SP
                # load queue so loads and stores overlap (measured ~430GB/s
                # bidirectional vs ~236GB/s single-direction)
                nc.gpsimd.dma_start(
                    out=ys[0:80, row0 * W : row0 * W + lc], in_=xt[0:80, :lc]
                )
                nc.gpsimd.dma_start(
                    out=ys[80:112, row0 * W : row0 * W + lc],
                    in_=xt[96:128, :lc],
                )

            # issue ALL loads up-front so the in-order SP DMA queue never
            # head-blocks later loads behind mult-gated stores
            xts = []
            row0 = 0
            for rows in CHUNK_ROWS:
                lc = rows * W
                xt = px.tile([128, LMAX], f16, tag="xt")
                cols = slice(row0 * W, row0 * W + lc)
                nc.sync.dma_start(out=xt[0:80, :lc], in_=xs[0:80, cols])
                nc.sync.dma_start(out=xt[96:128, :lc], in_=xs[80:112, cols])
                xts.append(xt)
                row0 += rows

            pending = []
            row0 = 0
            for ci, rows in enumerate(CHUNK_ROWS):
                lc = rows * W
                r2, r4, r8 = rows // 2, rows // 4, rows // 8
                xt = xts[ci]
                msml = pmm.tile([80, LMAX // 4], f16, tag="msml")
                t1 = pt.tile([80, LMAX // 2], f16, tag="t1")
                sa = pt.tile([80, LMAX // 4], f16, tag="sa")
                t2 = pt.tile([80, LMAX // 8], f16, tag="t2")
                sb = pt.tile([80, LMAX // 16], f16, tag="sb")
                m4 = pt.tile([80, LMAX // 16], f16, tag="m4")
                t3 = pt.tile([80, LMAX // 32], f16, tag="t3")
                sc = pt.tile([80, LMAX // 64], f16, tag="sc")

                # relu [96:128] up-front: depends only on this chunk's load;
                # row-split ACT / GpSimd / DVE (GpSimd is otherwise idle and
                # off the mask->mult critical chain)
                sr = int(round(RELU_ACT_ROWS_FRAC * rows))
                sg = sr + int(round(RELU_GPS_ROWS_FRAC * rows))
                nc.scalar.activation(
                    out=xt[96:128, : sr * W], in_=xt[96:128, : sr * W],
                    func=AF.Relu,
                )
                if sg > sr:
                    nc.gpsimd.tensor_scalar(
                        xt[96:128, sr * W : sg * W], xt[96:128, sr * W : sg * W],
                        0.0, None, ALU.max,
                    )
                if sg < rows:
                    nc.vector.tensor_scalar(
                        xt[96:128, sg * W : lc], xt[96:128, sg * W : lc],
                        0.0, None, ALU.max,
                    )

                V = nc.vector

                def rowpair(src, dst, p0, p1, w, r):
                    """dst (r/2, w) = row-pair sums of src (r, w); packed."""
                    vr = src[p0:p1, : r * w].rearrange(
                        "p (r t a) -> p r t a", t=2, a=w
                    )
                    V.tensor_tensor(
                        out=dst[p0:p1, : r * w // 2].rearrange(
                            "p (r a) -> p r a", a=w
                        ),
                        in0=vr[:, :, 0, :], in1=vr[:, :, 1, :], op=ALU.add)

                def halfadd(src, dst, p0, p1, w, r):
                    """dst (r, w/2) = src[:, :w/2] + src[:, w/2:]; packed."""
                    vh = src[p0:p1, : r * w].rearrange(
                        "p (r h a) -> p r h a", h=2, a=w // 2
                    )
                    V.tensor_tensor(
                        out=dst[p0:p1, : r * w // 2].rearrange(
                            "p (r a) -> p r a", a=w // 2
                        ),
                        in0=vh[:, :, 0, :], in1=vh[:, :, 1, :], op=ALU.add)

                # 2x2 sums [0:80]
                rowpair(xt, t1, 0, 80, W, rows)
                halfadd(t1, sa, 0, 80, W, r2)
                nc.vector.tensor_scalar(
                    msml[0:32, : lc // 4], sa[0:32, : lc // 4], 0.0, None,
                    ALU.is_gt,
                )
                # 4x4 sums ([0:32] wasted but merged op is free-size bound)
                rowpair(sa, t2, 0, 80, 96, r2)
                halfadd(t2, sb, 0, 80, 96, r4)
                nc.vector.tensor_scalar(
                    m4[32:64, : lc // 16], sb[32:64, : lc // 16], 0.0, None,
                    ALU.is_gt,
                )
                # 8x8 sums (g8 only)
                rowpair(sb, t3, 64, 80, 48, r4)
                halfadd(t3, sc, 64, 80, 48, r8)
                nc.vector.tensor_scalar(
                    sc[64:80, : lc // 64], sc[64:80, : lc // 64], 0.0, None,
                    ALU.is_gt,
                )

                # g4 expansion: msml[v] = m4[v % 48], per row-parity (ACT)
                vg4 = msml[32:64, : lc // 4].rearrange(
                    "p (r d a) -> p r d a", d=2, a=96
                )
                m4v = m4[32:64, : lc // 16].rearrange("p (r t) -> p r t", t=48)
                for dr in range(2):
                    nc.scalar.copy(
                        out=vg4[:, :, dr, :].rearrange(
                            "p r (u t) -> p r u t", t=48
                        ),
                        in_=m4v.unsqueeze(2).broadcast_to([32, r4, 2, 48]),
                    )
                # g8 expansion: msml[v] = m8[v % 24], per row-parity (ACT)
                vg8 = msml[64:80, : lc // 4].rearrange(
                    "p (r d a) -> p r d a", d=4, a=96
                )
                m8v = sc[64:80, : lc // 64].rearrange("p (r t) -> p r t", t=24)
                for dr in range(4):
                    nc.scalar.copy(
                        out=vg8[:, :, dr, :].rearrange(
                            "p r (q t) -> p r q t", t=24
                        ),
                        in_=m8v.unsqueeze(2).broadcast_to([16, r8, 4, 24]),
                    )

                pending.append((xt, msml, row0, rows))
                if len(pending) > PIPE_DEPTH:
                    emit_mult(*pending.pop(0))
                row0 += rows

            while pending:
                emit_mult(*pending.pop(0))

    return nc


def make_in_maps(activation: np.ndarray) -> list[dict]:
    """Per-core device inputs: [112, HW] fp16, channels PERM2, per-group
    column de-interleave, (c n) row order."""
    gi = np.arange(C2)[:, None, None]
    ci = COLP[:, None, :]
    maps = []
    for k in range(NCORES):
        a = activation[k * NB : (k + 1) * NB][:, PERM2]  # [2, 56, H, W]
        a = a[:, gi, np.arange(H)[None, :, None], ci]  # per-group col perm
        a = np.ascontiguousarray(a.transpose(1, 0, 2, 3).astype(np.float16))
        maps.append({"x": a.reshape(DROWS, HW)})
    return maps


def kernel(activation: np.ndarray) -> np.ndarray:
    from concourse import bass_utils

    activation = np.asarray(activation)
    assert activation.shape == (N, C, H, W) and activation.dtype == np.float32

    if "nc" not in _CACHE:
        _CACHE["nc"] = _build_nc()
    nc = _CACHE["nc"]

    in_maps = make_in_maps(activation)
    res = bass_utils.run_bass_kernel_spmd(nc, in_maps, core_ids=list(range(NCORES)))
    out = np.empty((N, C, H, W), dtype=np.float32)
    gi = np.arange(C2)[:, None, None]
    ici = ICOLP[:, None, :]
    for k in range(NCORES):
        yk = (
            np.asarray(res.results[k]["y"])
            .reshape(C2, NB, H, W)
            .transpose(1, 0, 2, 3)
            .astype(np.float32)
        )
        yk = yk[:, gi, np.arange(H)[None, :, None], ici]  # undo col perm
        out[k * NB : (k + 1) * NB, PERM2] = yk
    out[:, 56:64] = activation[:, 56:64]  # identity channels: exact f32
    return out


# revision 10
# speedup vs baseline: 2.8151x; 1.0145x over previous
"""BlockReLU Trainium2 kernel (8-core data-parallel over batch), fp16 device
I/O with per-group nested column de-interleaving.

Reference semantics (per [N, C, H, W] f32 input):
  channels  0:16  block (1,1): out = x * (x > 0)            == relu(x)
  channels 16:32  block (2,2): out = x * (mean_2x2(x) > 0)
  channels 32:48  block (4,4): out = x * (mean_4x4(x) > 0)
  channels 48:56  block (8,8): out = x * (mean_8x8(x) > 0)
  channels 56:64  identity

sign(mean) == sign(sum), so block sums replace means.  Device compute is
fp16 (correctness gate is rel_err < 2e-2; fp16 end-to-end measures
~2.7e-4), which halves HBM traffic and enables DVE 16-bit perf modes.
Identity channels are passed through on the host (exact f32).

Layout: per core the batch shard is packed to [112, HW] fp16, one
(channel, image) per dram row / SBUF partition:

  partitions  0:32   block (2,2) channels (c 16:32)
  partitions 32:64   block (4,4) channels (c 32:48)
  partitions 64:80   block (8,8) channels (c 48:56)
  partitions 96:128  block (1,1) channels (c  0:16)  (dram rows 80:112)

Within each row, columns are permuted host-side so that EVERY device op
is innermost-packed (measured: packed TT 0.60 ns/elem vs 1.1+ strided;
TS packed 0.34; broadcast only on non-innermost dims):

  level 1 (all block groups): [even cols | odd cols] — column-pair sums
    become two half-row adds, and the masked multiply becomes
    (row-pair, half, 96) with the quarter-res mask broadcast over the
    *middle* dim, innermost packed.
  level 2 (g4, g8): pair-index de-interleaved again — 4x4 column sums
    are again half-adds, and g4 mask expansion is msml[v] = m4[v % 48]
    (pure packed copies).
  level 3 (g8): once more — 8x8 sums are half-adds and g8 expansion is
    msml[v] = m8[v % 24].

Work split, balanced to the measured rates (DVE 0.6/0.34, ACT 0.97
pattern-insensitive, GpSimd 2.1 flat, DMA ~40-50us for 16.5 MB/core):
  DVE: all block-sum adds, is_gt masks, multiply row-parity 0, ~30% of
    relu rows (tensor_scalar max).
  ACT: mask expansions (copies) + ~70% of relu rows.
  GpSimd: multiply row-parity 1.
  DMA via SP HWDGE, 2 transfers per chunk per direction.
"""

import json
import re

import numpy as np

N, C, H, W = 16, 64, 192, 192
NCORES = 8
NB = N // NCORES  # batch per core
HW = H * W
C2 = 56  # channels computed on device (identity 56:64 handled on host)
DROWS = C2 * NB  # dram rows per core

CHUNK_ROWS = [8, 24, 32, 32, 32, 32, 24, 8]  # rows per chunk (each mult of 8)
# 32 rows -> 12.3KB per-partition DMA descriptors (18.4KB descriptors
# measured ~2x slower per byte on this part)
assert sum(CHUNK_ROWS) == H

XT_BUFS = len(CHUNK_ROWS)  # all loads are issued up-front
MSML_BUFS = len(CHUNK_ROWS)  # one per chunk: no ring-reuse waits
TMP_BUFS = 3
PIPE_DEPTH = 3

# relu row split: ScalarE / GpSimd (off the mask-mult critical chain) / DVE
RELU_ACT_ROWS_FRAC = 0.65
RELU_GPS_ROWS_FRAC = 0.0  # GpSimd tensor_scalar measured ~15ns/elem: unusable
MULT_DH1_GPS_FRAC = 0.0  # keep GpSimd off the mult critical chain

# device channel order: g2, g4, g8, then relu block
PERM2 = list(range(16, 56)) + list(range(0, 16))


def _deint(a):
    return np.concatenate([a[0::2], a[1::2]])


def _col_perms():
    """Per-group column permutation: packed[:, u] = x[:, perm[u]]."""
    ar = np.arange(W)
    p1 = _deint(ar)  # level 1: [even | odd]
    q = _deint(np.arange(W // 2))
    p2 = np.concatenate([p1[:96][q], p1[96:][q]])  # level 2 within halves
    r = _deint(np.arange(48))
    q8 = np.concatenate([2 * r, 2 * r + 1])
    p3 = np.concatenate([p1[:96][q8], p1[96:][q8]])  # level 3 variant
    return {
        "g2": p1,
        "g4": p2,
        "g8": p3,
        "g1": ar,
    }


_PC = _col_perms()
# per-device-channel-row column perm: rows 0:16 g2, 16:32 g4, 32:40 g8,
# 40:56 g1
_ROWGROUP = ["g2"] * 16 + ["g4"] * 16 + ["g8"] * 8 + ["g1"] * 16
COLP = np.stack([_PC[g] for g in _ROWGROUP])  # [56, 192]
ICOLP = np.argsort(COLP, axis=1)  # inverse perms

_CACHE = {}


def _split_multi_waits(bir_json: bytes) -> bytes:
    """This walrus build rejects >1 sync-wait per instruction; hoist extra
    waits onto fresh single-wait NoOps on the same engine."""
    m = json.loads(bir_json)
    max_idx = 0
    for f in m.get("functions", []):
        for b in f.get("blocks", []):
            for ins in b.get("instructions", []):
                mt = re.match(r"I-(\d+)$", ins.get("name", ""))
                if mt:
                    max_idx = max(max_idx, int(mt.group(1)))
    next_idx = max_idx + 1
    for f in m.get("functions", []):
        for b in f.get("blocks", []):
            out = []
            for ins in b.get("instructions", []):
                si = ins.get("sync_info")
                waits = (si or {}).get("on_wait") or []
                if len(waits) > 1:
                    for w in waits[:-1]:
                        out.append(
                            {
                                "debug": ins.get("debug"),
                                "engine": ins["engine"],
                                "ins": [],
                                "name": f"I-{next_idx}",
                                "opcode": "NoOp",
                                "outs": [],
                                "sync_info": {"on_wait": [w], "on_update": []},
                            }
                        )
                        next_idx += 1
                    si["on_wait"] = [waits[-1]]
                out.append(ins)
            b["instructions"] = out
    return json.dumps(m).encode()


def _install_birpatch():
    import concourse.bass2jax as b2j
    import concourse.bass_utils as bu

    if getattr(bu, "_split_waits_installed", False):
        return
    orig = bu.compile_bir_kernel

    def compile_bir_kernel_split(bir_json, tmpdir, neff_name="file.neff"):
        return orig(_split_multi_waits(bir_json), tmpdir, neff_name)

    bu.compile_bir_kernel = compile_bir_kernel_split
    b2j.compile_bir_kernel = compile_bir_kernel_split
    bu._split_waits_installed = True


def _build_nc():
    import concourse.bass as bass
    import concourse.mybir as mybir
    from concourse.tile import TileContext

    _install_birpatch()

    f16 = mybir.dt.float16
    ALU = mybir.AluOpType
    AF = mybir.ActivationFunctionType

    nc = bass.Bass("TRN2", debug=False)
    xs = nc.dram_tensor("x", [DROWS, HW], f16, kind="ExternalInput").ap()
    ys = nc.dram_tensor("y", [DROWS, HW], f16, kind="ExternalOutput").ap()

    RMAX = max(CHUNK_ROWS)
    LMAX = RMAX * W

    with TileContext(nc) as tc:
        with (
            tc.tile_pool(name="xt", bufs=XT_BUFS) as px,
            tc.tile_pool(name="mm", bufs=MSML_BUFS) as pmm,
            tc.tile_pool(name="tmp", bufs=TMP_BUFS) as pt,
        ):

            def emit_mult(xt, msml, row0, rows):
                """masked multiply + store, PIPE_DEPTH chunks behind."""
                lc = rows * W
                r2 = rows // 2

                # masked multiply [0:80]: (r2, parity, half, 96), mask
                # broadcast over parity (sub-op dim) and half (middle dim)
                vx = xt[0:80, :lc].rearrange(
                    "p (r t h a) -> p r t h a", t=2, h=2, a=96
                )
                mbf = msml[0:80, : lc // 4].rearrange("p (r a) -> p r a", a=96)

                def mult(eng, dh, a, b):
                    if a >= b:
                        return
                    o = vx[:, a:b, dh, :, :]
                    m = (
                        mbf[:, a:b, :]
                        .unsqueeze(2)
                        .broadcast_to([80, b - a, 2, 96])
                    )
                    eng.tensor_tensor(out=o, in0=o, in1=m, op=ALU.mult)

                ug = int(round(MULT_DH1_GPS_FRAC * r2))
                mult(nc.vector, 0, 0, r2)
                if ug:
                    mult(nc.gpsimd, 1, 0, ug)
                mult(nc.vector, 1, ug, r2)

                # stores on the GpSimd SWDGE queue: independent of the SP
                # load queue so loads and stores overlap (measured ~430GB/s
                # bidirectional vs ~236GB/s single-direction)
                nc.gpsimd.dma_start(
                    out=ys[0:80, row0 * W : row0 * W + lc], in_=xt[0:80, :lc]
                )
                nc.gpsimd.dma_start(
                    out=ys[80:112, row0 * W : row0 * W + lc],
                    in_=xt[96:128, :lc],
                )

            # issue ALL loads up-front so the in-order SP DMA queue never
            # head-blocks later loads behind mult-gated stores
            xts = []
            row0 = 0
            for rows in CHUNK_ROWS:
                lc = rows * W
                xt = px.tile([128, LMAX], f16, tag="xt")
                cols = slice(row0 * W, row0 * W + lc)
                nc.sync.dma_start(out=xt[0:80, :lc], in_=xs[0:80, cols])
                nc.sync.dma_start(out=xt[96:128, :lc], in_=xs[80:112, cols])
                xts.append(xt)
                row0 += rows

            pending = []
            row0 = 0
            for ci, rows in enumerate(CHUNK_ROWS):
                lc = rows * W
                r2, r4, r8 = rows // 2, rows // 4, rows // 8
                xt = xts[ci]
                msml = pmm.tile([80, LMAX // 4], f16, tag="msml")
                t1 = pt.tile([80, LMAX // 2], f16, tag="t1")
                sa = pt.tile([80, LMAX // 4], f16, tag="sa")
                t2 = pt.tile([80, LMAX // 8], f16, tag="t2")
                sb = pt.tile([80, LMAX // 16], f16, tag="sb")
                m4 = pt.tile([80, LMAX // 16], f16, tag="m4")
                t3 = pt.tile([80, LMAX // 32], f16, tag="t3")
                sc = pt.tile([80, LMAX // 64], f16, tag="sc")

                # relu [96:128] up-front: depends only on this chunk's load;
                # row-split ACT / GpSimd / DVE (GpSimd is otherwise idle and
                # off the mask->mult critical chain)
                sr = int(round(RELU_ACT_ROWS_FRAC * rows))
                sg = sr + int(round(RELU_GPS_ROWS_FRAC * rows))
                nc.scalar.activation(
                    out=xt[96:128, : sr * W], in_=xt[96:128, : sr * W],
                    func=AF.Relu,
                )
                if sg > sr:
                    nc.gpsimd.tensor_scalar(
                        xt[96:128, sr * W : sg * W], xt[96:128, sr * W : sg * W],
                        0.0, None, ALU.max,
                    )
                if sg < rows:
                    nc.vector.tensor_scalar(
                        xt[96:128, sg * W : lc], xt[96:128, sg * W : lc],
                        0.0, None, ALU.max,
                    )

                V = nc.vector

                def rowpair(src, dst, p0, p1, w, r):
                    """dst (r/2, w) = row-pair sums of src (r, w); packed."""
                    vr = src[p0:p1, : r * w].rearrange(
                        "p (r t a) -> p r t a", t=2, a=w
                    )
                    V.tensor_tensor(
                        out=dst[p0:p1, : r * w // 2].rearrange(
                            "p (r a) -> p r a", a=w
                        ),
                        in0=vr[:, :, 0, :], in1=vr[:, :, 1, :], op=ALU.add)

                def halfadd(src, dst, p0, p1, w, r):
                    """dst (r, w/2) = src[:, :w/2] + src[:, w/2:]; packed."""
                    vh = src[p0:p1, : r * w].rearrange(
                        "p (r h a) -> p r h a", h=2, a=w // 2
                    )
                    V.tensor_tensor(
                        out=dst[p0:p1, : r * w // 2].rearrange(
                            "p (r a) -> p r a", a=w // 2
                        ),
                        in0=vh[:, :, 0, :], in1=vh[:, :, 1, :], op=ALU.add)

                # 2x2 sums [0:80]
                rowpair(xt, t1, 0, 80, W, rows)
                halfadd(t1, sa, 0, 80, W, r2)
                nc.vector.tensor_scalar(
                    msml[0:32, : lc // 4], sa[0:32, : lc // 4], 0.0, None,
                    ALU.is_gt,
                )
                # 4x4 sums ([0:32] wasted but merged op is free-size bound)
                rowpair(sa, t2, 0, 80, 96, r2)
                halfadd(t2, sb, 0, 80, 96, r4)
                nc.vector.tensor_scalar(
                    m4[32:64, : lc // 16], sb[32:64, : lc // 16], 0.0, None,
                    ALU.is_gt,
                )
                # 8x8 sums (g8 only)
                rowpair(sb, t3, 64, 80, 48, r4)
                halfadd(t3, sc, 64, 80, 48, r8)
                nc.vector.tensor_scalar(
                    sc[64:80, : lc // 64], sc[64:80, : lc // 64], 0.0, None,
                    ALU.is_gt,
                )

                # g4 expansion: msml[v] = m4[v % 48], per row-parity (ACT)
                vg4 = msml[32:64, : lc // 4].rearrange(
                    "p (r d a) -> p r d a", d=2, a=96
                )
                m4v = m4[32:64, : lc // 16].rearrange("p (r t) -> p r t", t=48)
                for dr in range(2):
                    nc.scalar.copy(
                        out=vg4[:, :, dr, :].rearrange(
                            "p r (u t) -> p r u t", t=48
                        ),
                        in_=m4v.unsqueeze(2).broadcast_to([32, r4, 2, 48]),
                    )
                # g8 expansion: msml[v] = m8[v % 24], per row-parity (ACT)
                vg8 = msml[64:80, : lc // 4].rearrange(
                    "p (r d a) -> p r d a", d=4, a=96
                )
                m8v = sc[64:80, : lc // 64].rearrange("p (r t) -> p r t", t=24)
                for dr in range(4):
                    nc.scalar.copy(
                        out=vg8[:, :, dr, :].rearrange(
                            "p r (q t) -> p r q t", t=24
                        ),
                        in_=m8v.unsqueeze(2).broadcast_to([16, r8, 4, 24]),
                    )

                pending.append((xt, msml, row0, rows))
                if len(pending) > PIPE_DEPTH:
                    emit_mult(*pending.pop(0))
                row0 += rows

            while pending:
                emit_mult(*pending.pop(0))

    return nc


def make_in_maps(activation: np.ndarray) -> list[dict]:
    """Per-core device inputs: [112, HW] fp16, channels PERM2, per-group
    column de-interleave, (c n) row order."""
    gi = np.arange(C2)[:, None, None]
    ci = COLP[:, None, :]
    maps = []
    for k in range(NCORES):
        a = activation[k * NB : (k + 1) * NB][:, PERM2]  # [2, 56, H, W]
        a = a[:, gi, np.arange(H)[None, :, None], ci]  # per-group col perm
        a = np.ascontiguousarray(a.transpose(1, 0, 2, 3).astype(np.float16))
        maps.append({"x": a.reshape(DROWS, HW)})
    return maps


def kernel(activation: np.ndarray) -> np.ndarray:
    from concourse import bass_utils

    activation = np.asarray(activation)
    assert activation.shape == (N, C, H, W) and activation.dtype == np.float32

    if "nc" not in _CACHE:
        _CACHE["nc"] = _build_nc()
    nc = _CACHE["nc"]

    in_maps = make_in_maps(activation)
    res = bass_utils.run_bass_kernel_spmd(nc, in_maps, core_ids=list(range(NCORES)))
    out = np.empty((N, C, H, W), dtype=np.float32)
    gi = np.arange(C2)[:, None, None]
    ici = ICOLP[:, None, :]
    for k in range(NCORES):
        yk = (
            np.asarray(res.results[k]["y"])
            .reshape(C2, NB, H, W)
            .transpose(1, 0, 2, 3)
            .astype(np.float32)
        )
        yk = yk[:, gi, np.arange(H)[None, :, None], ici]  # undo col perm
        out[k * NB : (k + 1) * NB, PERM2] = yk
    out[:, 56:64] = activation[:, 56:64]  # identity channels: exact f32
    return out
